# revision 1
# baseline (speedup 1.0000x reference)
"""DGCNN classifier forward pass on 8 Trainium2 NeuronCores (Bass/Tile).

Data-parallel over batch: 2 point clouds per core. Per sample:
  4 EdgeConv layers, each:
    - kNN scores via one augmented matmul: score[n,m] = <f_n,f_m> - ||f_m||^2/2
      (rank-equivalent to the reference's pairwise-distance top-k)
    - top-20 per row on the DVE via MAX8/MATCH_REPLACE cascades over
      index-packed scores (column index injected into the low 10 mantissa bits)
    - neighbor max-aggregation via GPSIMD ap_gather over u = Wn @ f, using
      monotonicity of the (positive-gamma) BN + LeakyReLU to commute max:
      h = lrelu(A*(max_k u[idx] + (Wx-Wn) @ f) + B)
  then the 1024-wide conv + max/mean pooling and the 3-layer MLP head.
"""
import numpy as np
from contextlib import ExitStack

import concourse.bass as bass
import concourse.bacc as bacc
import concourse.mybir as mybir
from concourse import tile

F32 = mybir.dt.float32
U32 = mybir.dt.uint32
U16 = mybir.dt.uint16
I16 = mybir.dt.int16
AF = mybir.ActivationFunctionType
ALU = mybir.AluOpType
AX = mybir.AxisListType

N = 1024
K = 20
EPS = 1e-5
NEG = -3.0e38
# (C, O, input location, output location) per edge-conv layer.
LAYERS = [(3, 64), (64, 64), (64, 128), (128, 256)]


def build_nc():
    nc = bacc.Bacc("TRN2", target_bir_lowering=False, debug=False)

    x_d = nc.dram_tensor("x", [2, 3, N], F32, kind="ExternalInput")
    w_d = {}
    for name, shape in [("w1", (64, 6)), ("w2", (64, 128)), ("w3", (128, 128)),
                        ("w4", (256, 256)), ("w5", (1024, 512)),
                        ("l1w", (512, 2048)), ("l2w", (256, 512)), ("l3w", (40, 256)),
                        ("l2b", (256,)), ("l3b", (40,))]:
        w_d[name] = nc.dram_tensor(name, list(shape), F32, kind="ExternalInput")
    for i, c in zip(range(1, 8), [64, 64, 128, 256, 1024, 512, 256]):
        w_d["bn%d" % i] = nc.dram_tensor("bn%d" % i, [4, c], F32, kind="ExternalInput")
    out_d = nc.dram_tensor("outT", [40, 2], F32, kind="ExternalOutput")

    with tile.TileContext(nc) as tc, ExitStack() as ctx:
        emit(nc, tc, ctx, x_d, w_d, out_d)
    nc.compile()
    return nc


def _stt_u32(nc, out, in0, imm, in1, op0, op1):
    """scalar_tensor_tensor with a uint32-typed immediate (bitwise-safe)."""
    eng = nc.vector
    return eng.add_instruction(mybir.InstTensorScalarPtr(
        name=nc.get_next_instruction_name(),
        is_scalar_tensor_tensor=True,
        op0=op0, op1=op1,
        ins=[eng.lower_ap(in0),
             mybir.ImmediateValue(dtype=U32, value=imm),
             eng.lower_ap(in1)],
        outs=[eng.lower_ap(out)],
    ))


def _ts_u32(nc, out, in0, imm, op0):
    """tensor_scalar with a uint32-typed immediate."""
    eng = nc.vector
    return eng.add_instruction(mybir.InstTensorScalarPtr(
        name=nc.get_next_instruction_name(),
        op0=op0, op1=ALU.bypass,
        ins=[eng.lower_ap(in0),
             mybir.ImmediateValue(dtype=U32, value=imm)],
        outs=[eng.lower_ap(out)],
    ))


def _bn_affine(nc, pool, bnT, tag):
    """bnT: [C<=128, 4] tile AP (cols g,b,m,v) -> (A, B) [C,1] tiles."""
    Cc = bnT.shape[0]
    A = pool.tile([Cc, 1], F32, tag=tag + "A")
    B = pool.tile([Cc, 1], F32, tag=tag + "B")
    t = pool.tile([Cc, 1], F32, tag=tag + "t")
    nc.vector.tensor_scalar(out=t[:], in0=bnT[:, 3:4], scalar1=EPS, scalar2=None,
                            op0=ALU.add)
    nc.vector.reciprocal(out=t[:], in_=t[:])
    nc.scalar.activation(out=t[:], in_=t[:], func=AF.Sqrt)
    nc.vector.tensor_tensor(out=A[:], in0=bnT[:, 0:1], in1=t[:], op=ALU.mult)
    nc.vector.tensor_tensor(out=t[:], in0=bnT[:, 2:3], in1=A[:], op=ALU.mult)
    nc.vector.tensor_tensor(out=B[:], in0=bnT[:, 1:2], in1=t[:], op=ALU.subtract)
    return A, B


def emit(nc, tc, ctx, x_d, w_d, out_d):
    wp = ctx.enter_context(tc.tile_pool(name="wp", bufs=1))
    cat = ctx.enter_context(tc.tile_pool(name="cat", bufs=1))
    work = ctx.enter_context(tc.tile_pool(name="work", bufs=2))
    feat = ctx.enter_context(tc.tile_pool(name="feat", bufs=1))
    small = ctx.enter_context(tc.tile_pool(name="small", bufs=2))
    psS = ctx.enter_context(tc.tile_pool(name="psS", bufs=2, space="PSUM"))
    psX = ctx.enter_context(tc.tile_pool(name="psX", bufs=1, space="PSUM"))
    psU = ctx.enter_context(tc.tile_pool(name="psU", bufs=1, space="PSUM"))
    dram = ctx.enter_context(tc.tile_pool(name="dram", bufs=1, space="DRAM"))

    # ---------------- constants ----------------
    iota = wp.tile([128, N], U32, tag="iota")
    nc.gpsimd.iota(iota[:], pattern=[[1, N]], base=0, channel_multiplier=0)
    ones = wp.tile([128, 1], F32, tag="ones")
    nc.vector.memset(ones[:], 1.0)
    cm05 = wp.tile([1, 128], F32, tag="cm05")
    nc.vector.memset(cm05[:], -0.5)

    # ---------------- weights (transposed loads) ----------------
    def tload(dst, src_ap):
        nc.sync.dma_start(out=dst, in_=src_ap)

    # per-layer Wn^T / Wx^T halves, each its own DMA; wxm built so every
    # instruction depends on at most one DMA (HW sync-wait slot limit).
    wn, wxm = [], []
    for nm, (O_, C2) in [("w1", (64, 6)), ("w2", (64, 128)), ("w3", (128, 128)),
                         ("w4", (256, 256))]:
        Cl = C2 // 2
        tn = wp.tile([Cl, O_], F32, tag=nm + "Tn", name=nm + "Tn")
        tload(tn[:], w_d[nm][:, 0:Cl].rearrange("o c -> c o"))
        tx = wp.tile([Cl, O_], F32, tag=nm + "Tx", name=nm + "Tx")
        tload(tx[:], w_d[nm][:, Cl:C2].rearrange("o c -> c o"))
        m = wp.tile([Cl, O_], F32, tag=nm + "wxm", name=nm + "wxm")
        nc.vector.tensor_copy(out=m[:], in_=tx[:])
        nc.vector.tensor_tensor(out=m[:], in0=m[:], in1=tn[:], op=ALU.subtract)
        wn.append(tn[:])
        wxm.append(m[:])

    w5T = []
    for ci in range(4):
        t = wp.tile([128, 1024], F32, tag=f"w5T{ci}")
        tload(t[:], w_d["w5"][:, ci * 128:(ci + 1) * 128].rearrange("o c -> c o"))
        w5T.append(t)
    l1wT = []
    for ci in range(16):
        t = wp.tile([128, 512], F32, tag=f"l1wT{ci}")
        tload(t[:], w_d["l1w"][:, ci * 128:(ci + 1) * 128].rearrange("o c -> c o"))
        l1wT.append(t)
    l2wT = []
    for ci in range(4):
        t = wp.tile([128, 256], F32, tag=f"l2wT{ci}")
        tload(t[:], w_d["l2w"][:, ci * 128:(ci + 1) * 128].rearrange("o c -> c o"))
        l2wT.append(t)
    l3wT = []
    for ci in range(2):
        t = wp.tile([128, 40], F32, tag=f"l3wT{ci}")
        tload(t[:], w_d["l3w"][:, ci * 128:(ci + 1) * 128].rearrange("o c -> c o"))
        l3wT.append(t)

    # bn affines
    bnAB = {}
    for i, c in zip(range(1, 5), [64, 64, 128, 256]):
        nch = (c + 127) // 128
        As, Bs = [], []
        for ch in range(nch):
            cc = min(128, c - ch * 128)
            bnT = wp.tile([cc, 4], F32, tag=f"bnT{i}_{ch}")
            tload(bnT[:], w_d["bn%d" % i][:, ch * 128:ch * 128 + cc].rearrange("f c -> c f"))
            A, B = _bn_affine(nc, wp, bnT, f"bn{i}_{ch}")
            As.append(A); Bs.append(B)
        bnAB[i] = (As, Bs)
    A5 = wp.tile([128, 8], F32, tag="A5")
    B5 = wp.tile([128, 8], F32, tag="B5")
    for ch in range(8):
        bnT = wp.tile([128, 4], F32, tag=f"bnT5_{ch}")
        tload(bnT[:], w_d["bn5"][:, ch * 128:(ch + 1) * 128].rearrange("f c -> c f"))
        A, B = _bn_affine(nc, wp, bnT, f"bn5_{ch}")
        nc.vector.tensor_copy(out=A5[:, ch:ch + 1], in_=A[:])
        nc.vector.tensor_copy(out=B5[:, ch:ch + 1], in_=B[:])
    A6 = wp.tile([128, 4], F32, tag="A6")
    B6 = wp.tile([128, 4], F32, tag="B6")
    for ch in range(4):
        bnT = wp.tile([128, 4], F32, tag=f"bnT6_{ch}")
        tload(bnT[:], w_d["bn6"][:, ch * 128:(ch + 1) * 128].rearrange("f c -> c f"))
        A, B = _bn_affine(nc, wp, bnT, f"bn6_{ch}")
        nc.vector.tensor_copy(out=A6[:, ch:ch + 1], in_=A[:])
        nc.vector.tensor_copy(out=B6[:, ch:ch + 1], in_=B[:])
    A7 = wp.tile([128, 2], F32, tag="A7")
    B7 = wp.tile([128, 2], F32, tag="B7")
    for ch in range(2):
        bnT = wp.tile([128, 4], F32, tag=f"bnT7_{ch}")
        tload(bnT[:], w_d["bn7"][:, ch * 128:(ch + 1) * 128].rearrange("f c -> c f"))
        A, B = _bn_affine(nc, wp, bnT, f"bn7_{ch}")
        # fold l2b: B7' = A7*l2b + B7
        l2bT = wp.tile([128, 1], F32, tag=f"l2bT{ch}")
        tload(l2bT[:], w_d["l2b"][ch * 128:(ch + 1) * 128].rearrange("(p o) -> p o", o=1))
        t = wp.tile([128, 1], F32, tag=f"b7f{ch}")
        nc.vector.tensor_tensor(out=t[:], in0=A[:], in1=l2bT[:], op=ALU.mult)
        nc.vector.tensor_tensor(out=t[:], in0=B[:], in1=t[:], op=ALU.add)
        nc.vector.tensor_copy(out=A7[:, ch:ch + 1], in_=A[:])
        nc.vector.tensor_copy(out=B7[:, ch:ch + 1], in_=t[:])
    l3bT = wp.tile([40, 1], F32, tag="l3bT")
    tload(l3bT[:], w_d["l3b"][:].rearrange("(p o) -> p o", o=1))

    # ---------------- per-sample feature tiles ----------------
    # cat layout per sample: catA rows 0:64 = h1, 64:128 = h2; catB = h3;
    # catC/catD = h4 chunks.  All [128, 1024].
    cats = []
    for s in range(2):
        cats.append([cat.tile([128, N], F32, tag=f"cat{t}_{s}", name=f"cat{t}_{s}") for t in "ABCD"])
    xT = []
    for s in range(2):
        t = cat.tile([3, N], F32, tag=f"xT{s}")
        nc.sync.dma_start(out=t[:], in_=x_d[s])
        xT.append(t)

    pooledT = cat.tile([128, 32], F32, tag="pooledT")

    for s in range(2):
        catA, catB, catC, catD = cats[s]
        fT_in = [xT[s][:], catA[0:64, :], catA[64:128, :], catB[:]]
        out_rows = [[catA[0:64, :]], [catA[64:128, :]], [catB[:]], [catC[:], catD[:]]]
        for li, (C, O) in enumerate(LAYERS):
            edge_conv_layer(nc, tc, work, feat, small, psS, psX, psU, dram,
                            s, li, C, O, fT_in[li],
                            wn[li], wxm[li], bnAB[li + 1],
                            out_rows[li], iota, ones, cm05)

        # ----- layer 5: 1024-wide conv + pooling -----
        catchunks = [catA, catB, catC, catD]
        for j in range(8):
            h5_ps = psS.tile([128, N], F32, tag="score")
            for ci in range(4):
                for f in range(0, N, 512):
                    nc.tensor.matmul(h5_ps[:, f:f + 512],
                                     w5T[ci][:, j * 128:(j + 1) * 128],
                                     catchunks[ci][:, f:f + 512],
                                     start=(ci == 0), stop=(ci == 3))
            h5_sb = work.tile([128, N], F32, tag="h5")
            sums = small.tile([128, 1], F32, tag="h5sum")
            nc.scalar.activation(out=h5_sb[:], in_=h5_ps[:], func=AF.Prelu,
                                 bias=B5[:, j:j + 1], scale=A5[:, j:j + 1],
                                 alpha=0.2, accum_out=sums[:])
            # mean -> pooled col (8+j)*2+s ; max -> pooled col j*2+s
            nc.scalar.activation(out=pooledT[:, (8 + j) * 2 + s:(8 + j) * 2 + s + 1],
                                 in_=sums[:], func=AF.Copy, scale=1.0 / N)
            nc.vector.tensor_reduce(out=pooledT[:, j * 2 + s:j * 2 + s + 1],
                                    in_=h5_sb[:], axis=AX.X, op=ALU.max)

    # ---------------- MLP head (both samples as free dim) ----------------
    h6T = work.tile([128, 4, 2], F32, tag="h6T")
    for j in range(4):
        h6_ps = psU.tile([128, 2], F32, tag="uv")
        for ci in range(16):
            nc.tensor.matmul(h6_ps[:], l1wT[ci][:, j * 128:(j + 1) * 128],
                             pooledT[:, ci * 2:ci * 2 + 2],
                             start=(ci == 0), stop=(ci == 15))
        nc.scalar.activation(out=h6T[:, j, :], in_=h6_ps[:], func=AF.Prelu,
                             bias=B6[:, j:j + 1], scale=A6[:, j:j + 1], alpha=0.2)
    h7T = work.tile([128, 2, 2], F32, tag="h7T")
    for j in range(2):
        h7_ps = psU.tile([128, 2], F32, tag="uv")
        for ci in range(4):
            nc.tensor.matmul(h7_ps[:], l2wT[ci][:, j * 128:(j + 1) * 128],
                             h6T[:, ci, :], start=(ci == 0), stop=(ci == 3))
        nc.scalar.activation(out=h7T[:, j, :], in_=h7_ps[:], func=AF.Prelu,
                             bias=B7[:, j:j + 1], scale=A7[:, j:j + 1], alpha=0.2)
    out_ps = psU.tile([40, 2], F32, tag="uv")
    for ci in range(2):
        nc.tensor.matmul(out_ps[:], l3wT[ci][:], h7T[:, ci, :],
                         start=(ci == 0), stop=(ci == 1))
    out_sb = small.tile([40, 2], F32, tag="out")
    nc.vector.tensor_scalar(out=out_sb[:], in0=out_ps[:], scalar1=l3bT[:],
                            scalar2=None, op0=ALU.add)
    nc.sync.dma_start(out=out_d[:], in_=out_sb[:])


def edge_conv_layer(nc, tc, work, feat, small, psS, psX, psU, dram,
                    s, li, C, O, fT, wnT, wxmT, bnab,
                    out_rows, iota, ones, cm05):
    As, Bs = bnab
    noc = (O + 127) // 128

    # xx = sum_c f^2 (via ones-matmul over partitions)
    sq = work.tile([C, N], F32, tag="sq")
    nc.vector.tensor_tensor(out=sq[:], in0=fT, in1=fT, op=ALU.mult)
    xx_ps = psX.tile([1, N], F32, tag="xx")
    for f in range(0, N, 512):
        nc.tensor.matmul(xx_ps[:, f:f + 512], ones[0:C, :], sq[:, f:f + 512],
                         start=True, stop=True)

    if C < 128:
        AUGP = 32 if C < 32 else C
        rhs_aug = work.tile([AUGP + 1, N], F32, tag="rhsaug")
        lhs_aug = work.tile([AUGP + 1, N], F32, tag="lhsaug")
        if AUGP != C:
            nc.vector.memset(rhs_aug[:], 0.0)
            nc.vector.memset(lhs_aug[:], 0.0)
        nc.scalar.activation(out=rhs_aug[0:C, :], in_=fT, func=AF.Copy)
        nc.scalar.activation(out=rhs_aug[AUGP:AUGP + 1, :], in_=xx_ps[:], func=AF.Copy)
        nc.scalar.activation(out=lhs_aug[0:C, :], in_=fT, func=AF.Copy)
        nc.vector.memset(lhs_aug[AUGP:AUGP + 1, :], -0.5)
        xx_sb = None
    else:
        rhs_aug = lhs_aug = None
        xx_sb = work.tile([1, N], F32, tag="xxsb")
        nc.scalar.activation(out=xx_sb[:], in_=xx_ps[:], func=AF.Copy)

    # u = Wn @ f, v = (Wx-Wn) @ f   (transposed [O, N]); rhs must share the
    # lhsT base partition, so use the base-0 copy in rhs_aug when C < 128.
    fT0 = rhs_aug[0:C, :] if C < 128 else fT
    uT_sb, vT_sb = [], []
    for oc in range(noc):
        ocw = min(128, O - oc * 128)
        ups = psU.tile([ocw, N], F32, tag="uv")
        for f in range(0, N, 512):
            nc.tensor.matmul(ups[:, f:f + 512], wnT[:, oc * 128:oc * 128 + ocw],
                             fT0[:, f:f + 512], start=True, stop=True)
        ut = feat.tile([ocw, N], F32, tag=f"u{oc}")
        nc.scalar.activation(out=ut[:], in_=ups[:], func=AF.Copy)
        uT_sb.append(ut)
        vps = psU.tile([ocw, N], F32, tag="uv")
        for f in range(0, N, 512):
            nc.tensor.matmul(vps[:, f:f + 512], wxmT[:, oc * 128:oc * 128 + ocw],
                             fT0[:, f:f + 512], start=True, stop=True)
        vt = feat.tile([ocw, N], F32, tag=f"v{oc}")
        nc.scalar.activation(out=vt[:], in_=vps[:], func=AF.Copy)
        vT_sb.append(vt)

    aggT = [feat.tile([min(128, O - oc * 128), N], F32, tag=f"agg{oc}", name=f"agg{oc}")
            for oc in range(noc)]

    for b in range(8):
        # ---- score tile [128, N] ----
        sc_ps = psS.tile([128, N], F32, tag="score")
        for f in range(0, N, 512):
            if C < 128:
                nc.tensor.matmul(sc_ps[:, f:f + 512],
                                 lhs_aug[:, b * 128:(b + 1) * 128],
                                 rhs_aug[:, f:f + 512], start=True, stop=True)
            else:
                nc.tensor.matmul(sc_ps[:, f:f + 512], fT[:, b * 128:(b + 1) * 128],
                                 fT[:, f:f + 512], start=True, stop=False)
                nc.tensor.matmul(sc_ps[:, f:f + 512], cm05[:],
                                 xx_sb[:, f:f + 512], start=False, stop=True)
        # ---- pack + top-20 cascade ----
        packed = work.tile([128, N], U32, tag="packed")
        _stt_u32(nc, packed[:], sc_ps[:].bitcast(U32), 0xFFFFFC00, iota[:],
                 ALU.bitwise_and, ALU.bitwise_or)
        packf = packed[:].bitcast(F32)
        top24 = small.tile([128, 24], F32, tag="top24")
        nc.vector.max(top24[:, 0:8], packf)
        nc.vector.match_replace(packf, top24[:, 0:8], packf, imm_value=NEG)
        nc.vector.max(top24[:, 8:16], packf)
        nc.vector.match_replace(packf, top24[:, 8:16], packf, imm_value=NEG)
        nc.vector.max(top24[:, 16:24], packf)
        idx32 = small.tile([128, 32], U32, tag="idx32")
        _ts_u32(nc, idx32[:, 0:20], top24[:, 0:20].bitcast(U32), 0x3FF,
                ALU.bitwise_and)
        nc.vector.tensor_copy(out=idx32[:, 20:32], in_=idx32[:, 0:12])
        idxP = small.tile([128, 32], U16, tag="idxP")
        nc.vector.tensor_copy(out=idxP[:], in_=idx32[:])
        # ---- wrap roundtrip (3 DMAs) ----
        scrW = dram.tile([4096], U16, tag="scrW")
        nc.sync.dma_start(
            out=scrW[:].rearrange("(j1 n j0) -> n j1 j0", j1=16, j0=2),
            in_=idxP[:])
        scr2 = dram.tile([8, 4096], U16, tag="scr2")
        nc.scalar.dma_start(out=scr2[:].rearrange("a q -> (a q)"),
                            in_=scrW[:].unsqueeze(0).broadcast_to([8, 4096]))
        idxw = small.tile([128, 256], I16, tag="idxw")
        nc.sync.dma_start(
            out=idxw[:],
            in_=scr2[:].bitcast(I16).rearrange("a q -> (a q)").rearrange("(p q) -> p q", p=128))
        # ---- gather + reduce (two 64-point halves) ----
        for oc in range(noc):
            ocw = min(128, O - oc * 128)
            for h in range(2):
                g = work.tile([ocw, 2048], F32, tag="g")
                nc.gpsimd.ap_gather(g[:], uT_sb[oc][:], idxw[0:ocw, h * 128:(h + 1) * 128],
                                    channels=ocw, num_elems=N, d=1, num_idxs=2048)
                gv = g[:].rearrange("o (n run pos) -> o n run pos", run=2, pos=16)[:, :, :, 0:10]
                nc.vector.tensor_reduce(
                    out=aggT[oc][:, b * 128 + h * 64: b * 128 + (h + 1) * 64],
                    in_=gv, axis=AX.XY, op=ALU.max)

    # ---- h = lrelu(A*(agg + v) + B) -> cat rows ----
    for oc in range(noc):
        nc.vector.tensor_tensor(out=aggT[oc][:], in0=aggT[oc][:], in1=vT_sb[oc][:],
                                op=ALU.add)
        nc.scalar.activation(out=out_rows[oc], in_=aggT[oc][:], func=AF.Prelu,
                             bias=Bs[oc][:], scale=As[oc][:], alpha=0.2)


_NC_CACHE = []


def kernel(**inputs):
    """Full-batch entry: shard 16 samples over 8 cores (2 each), run SPMD."""
    from concourse.bass_utils import run_bass_kernel_spmd

    if not _NC_CACHE:
        _NC_CACHE.append(build_nc())
    nc = _NC_CACHE[0]

    x = np.ascontiguousarray(inputs["x"], dtype=np.float32)
    base = {k: np.ascontiguousarray(v, dtype=np.float32)
            for k, v in inputs.items() if k != "x"}
    cores = list(range(8))
    in_maps = [dict(base, x=np.ascontiguousarray(x[2 * c:2 * c + 2])) for c in cores]
    res = run_bass_kernel_spmd(nc, in_maps, cores).results
    out = np.concatenate([np.ascontiguousarray(r["outT"]).T for r in res], axis=0)
    return out.astype(np.float32)



# revision 5
# speedup vs baseline: 1.0580x; 1.0580x over previous
"""DGCNN classifier forward pass on 8 Trainium2 NeuronCores (Bass/Tile).

Data-parallel over batch: 2 point clouds per core. Per sample:
  4 EdgeConv layers, each:
    - kNN scores via one augmented matmul: score[n,m] = <f_n,f_m> - ||f_m||^2/2
      (rank-equivalent to the reference's pairwise-distance top-k)
    - top-20 per row on the DVE via MAX8/MATCH_REPLACE cascades over
      index-packed scores (column index injected into the low 10 mantissa bits)
    - index redistribution for ap_gather done fully on-chip: PE transpose of
      the [128 rows, 32 slots] index tile + two 0/1 selection matmuls that
      place slot (2*(p%16)+j0) of every row on partition p, so each GPSIMD
      core sees all of its rows' neighbor indices in its own 16 partitions.
    - neighbor max-aggregation via GPSIMD ap_gather over u = Wn @ f, using
      monotonicity of the (positive-gamma) BN + LeakyReLU to commute max:
      h = lrelu(A*(max_k u[idx] + (Wx-Wn) @ f) + B)
  then the 1024-wide conv + max/mean pooling and the 3-layer MLP head.

Weights are DMA'd in natural (contiguous) layout and transposed on-chip by
the tensor engine; per-layer work is software-pipelined (3 stages per
128-row tile) to keep the DVE cascade saturated.
"""
import numpy as np
from contextlib import ExitStack

import concourse.bass as bass
import concourse.bacc as bacc
import concourse.mybir as mybir
from concourse import tile
from concourse import masks

F32 = mybir.dt.float32
U32 = mybir.dt.uint32
U16 = mybir.dt.uint16
I16 = mybir.dt.int16
AF = mybir.ActivationFunctionType
ALU = mybir.AluOpType
AX = mybir.AxisListType

N = 1024
K = 20
EPS = 1e-5
NEG = -3.0e38
LAYERS = [(3, 64), (64, 64), (64, 128), (128, 256)]


def build_nc():
    nc = bacc.Bacc("TRN2", target_bir_lowering=False, debug=False)

    x_d = nc.dram_tensor("x", [2, 3, N], F32, kind="ExternalInput")
    w_d = {}
    for name, shape in [("w1", (64, 6)), ("w2", (64, 128)), ("w3", (128, 128)),
                        ("w4", (256, 256)), ("w5", (1024, 512)),
                        ("l1w", (512, 2048)), ("l2w", (256, 512)), ("l3w", (40, 256)),
                        ("l2b", (256,)), ("l3b", (40,))]:
        w_d[name] = nc.dram_tensor(name, list(shape), F32, kind="ExternalInput")
    for i, c in zip(range(1, 8), [64, 64, 128, 256, 1024, 512, 256]):
        w_d["bn%d" % i] = nc.dram_tensor("bn%d" % i, [4, c], F32, kind="ExternalInput")
    out_d = nc.dram_tensor("outT", [40, 2], F32, kind="ExternalOutput")

    with tile.TileContext(nc) as tc, ExitStack() as ctx:
        emit(nc, tc, ctx, x_d, w_d, out_d)
    nc.compile()
    return nc


def _stt_u32(eng, nc, out, in0, imm, in1, op0, op1):
    """scalar_tensor_tensor with a uint32-typed immediate (bitwise-safe)."""
    return eng.add_instruction(mybir.InstTensorScalarPtr(
        name=nc.get_next_instruction_name(),
        is_scalar_tensor_tensor=True,
        op0=op0, op1=op1,
        ins=[eng.lower_ap(in0),
             mybir.ImmediateValue(dtype=U32, value=imm),
             eng.lower_ap(in1)],
        outs=[eng.lower_ap(out)],
    ))


def _ts_u32(eng, nc, out, in0, imm, op0):
    """tensor_scalar with a uint32-typed immediate."""
    return eng.add_instruction(mybir.InstTensorScalarPtr(
        name=nc.get_next_instruction_name(),
        op0=op0, op1=ALU.bypass,
        ins=[eng.lower_ap(in0),
             mybir.ImmediateValue(dtype=U32, value=imm)],
        outs=[eng.lower_ap(out)],
    ))


def _bn_affine(nc, pool, bnT, tag):
    """bnT: [C<=128, 4] tile AP (cols g,b,m,v) -> (A, B) [C,1] tiles."""
    Cc = bnT.shape[0]
    A = pool.tile([Cc, 1], F32, tag=tag + "A")
    B = pool.tile([Cc, 1], F32, tag=tag + "B")
    t = pool.tile([Cc, 1], F32, tag=tag + "t")
    nc.vector.tensor_scalar(out=t[:], in0=bnT[:, 3:4], scalar1=EPS, scalar2=None,
                            op0=ALU.add)
    nc.vector.reciprocal(out=t[:], in_=t[:])
    nc.scalar.activation(out=t[:], in_=t[:], func=AF.Sqrt)
    nc.vector.tensor_tensor(out=A[:], in0=bnT[:, 0:1], in1=t[:], op=ALU.mult)
    nc.vector.tensor_tensor(out=t[:], in0=bnT[:, 2:3], in1=A[:], op=ALU.mult)
    nc.vector.tensor_tensor(out=B[:], in0=bnT[:, 1:2], in1=t[:], op=ALU.subtract)
    return A, B


class Ctx:
    pass


def emit(nc, tc, ctx, x_d, w_d, out_d):
    g = Ctx()
    g.nc = nc
    g.wp = ctx.enter_context(tc.tile_pool(name="wp", bufs=1))
    g.nat = ctx.enter_context(tc.tile_pool(name="nat", bufs=2))
    g.cat = ctx.enter_context(tc.tile_pool(name="cat", bufs=1))
    g.feat = ctx.enter_context(tc.tile_pool(name="feat", bufs=2))
    g.aug = ctx.enter_context(tc.tile_pool(name="aug", bufs=1))
    g.sqp = ctx.enter_context(tc.tile_pool(name="sqp", bufs=1))
    g.scp = ctx.enter_context(tc.tile_pool(name="scp", bufs=2))
    g.gbuf = ctx.enter_context(tc.tile_pool(name="gbuf", bufs=1))
    g.small = ctx.enter_context(tc.tile_pool(name="small", bufs=2))
    g.psS = ctx.enter_context(tc.tile_pool(name="psS", bufs=2, space="PSUM"))
    g.psU = ctx.enter_context(tc.tile_pool(name="psU", bufs=1, space="PSUM"))
    g.psI = ctx.enter_context(tc.tile_pool(name="psI", bufs=1, space="PSUM"))
    g.psW = ctx.enter_context(tc.tile_pool(name="psW", bufs=1, space="PSUM"))
    wp = g.wp

    # ---------------- constants ----------------
    iota = wp.tile([128, N], U32, tag="iota")
    nc.gpsimd.iota(iota[:], pattern=[[1, N]], base=0, channel_multiplier=0)
    ident = wp.tile([128, 128], F32, tag="ident")
    masks.make_identity(nc, ident[:])
    cm05 = wp.tile([1, 128], F32, tag="cm05")
    nc.vector.memset(cm05[:], -0.5)
    # selection matrices B_j0 [32, 128]: B[k, p] = (k == 2*(p%16)+j0)
    rowk = wp.tile([32, 128], U32, tag="rowk")
    nc.gpsimd.iota(rowk[:], pattern=[[0, 128]], base=0, channel_multiplier=1)
    g.Bsel = []
    for j0 in range(2):
        colv = wp.tile([32, 128], U32, tag=f"colv{j0}")
        nc.gpsimd.iota(colv[:], pattern=[[0, 8], [2, 16]], base=j0,
                       channel_multiplier=0)
        Bj = wp.tile([32, 128], F32, tag=f"Bsel{j0}")
        nc.vector.tensor_tensor(out=Bj[:], in0=rowk[:], in1=colv[:],
                                op=ALU.is_equal)
        g.Bsel.append(Bj)
    g.iota = iota
    g.ident = ident
    g.cm05 = cm05

    # ---------------- small DMA loads (sync queue) ----------------
    def tload(dst, src_ap):
        nc.sync.dma_start(out=dst, in_=src_ap)

    xT = []
    for s in range(2):
        t = g.cat.tile([3, N], F32, tag=f"xT{s}")
        tload(t[:], x_d[s])
        xT.append(t)

    # w1 halves: tiny, element-level transpose DMA is fine
    wn1 = wp.tile([3, 64], F32, tag="wn1")
    tload(wn1[:], w_d["w1"][:, 0:3].rearrange("o c -> c o"))
    wx1 = wp.tile([3, 64], F32, tag="wx1")
    tload(wx1[:], w_d["w1"][:, 3:6].rearrange("o c -> c o"))
    wxm1 = wp.tile([3, 64], F32, tag="wxm1")
    nc.vector.tensor_copy(out=wxm1[:], in_=wx1[:])
    nc.vector.tensor_tensor(out=wxm1[:], in0=wxm1[:], in1=wn1[:], op=ALU.subtract)

    # bn params (small transposed loads) + affines
    bnAB = {}
    for i, c in zip(range(1, 5), [64, 64, 128, 256]):
        nch = (c + 127) // 128
        As, Bs = [], []
        for ch in range(nch):
            cc = min(128, c - ch * 128)
            bnT = wp.tile([cc, 4], F32, tag=f"bnT{i}_{ch}")
            tload(bnT[:], w_d["bn%d" % i][:, ch * 128:ch * 128 + cc].rearrange("f c -> c f"))
            A, B = _bn_affine(nc, wp, bnT, f"bn{i}_{ch}")
            As.append(A)
            Bs.append(B)
        bnAB[i] = (As, Bs)
    A5 = wp.tile([128, 8], F32, tag="A5")
    B5 = wp.tile([128, 8], F32, tag="B5")
    for ch in range(8):
        bnT = wp.tile([128, 4], F32, tag=f"bnT5_{ch}")
        tload(bnT[:], w_d["bn5"][:, ch * 128:(ch + 1) * 128].rearrange("f c -> c f"))
        A, B = _bn_affine(nc, wp, bnT, f"bn5_{ch}")
        nc.vector.tensor_copy(out=A5[:, ch:ch + 1], in_=A[:])
        nc.vector.tensor_copy(out=B5[:, ch:ch + 1], in_=B[:])
    A6 = wp.tile([128, 4], F32, tag="A6")
    B6 = wp.tile([128, 4], F32, tag="B6")
    for ch in range(4):
        bnT = wp.tile([128, 4], F32, tag=f"bnT6_{ch}")
        tload(bnT[:], w_d["bn6"][:, ch * 128:(ch + 1) * 128].rearrange("f c -> c f"))
        A, B = _bn_affine(nc, wp, bnT, f"bn6_{ch}")
        nc.vector.tensor_copy(out=A6[:, ch:ch + 1], in_=A[:])
        nc.vector.tensor_copy(out=B6[:, ch:ch + 1], in_=B[:])
    A7 = wp.tile([128, 2], F32, tag="A7")
    B7 = wp.tile([128, 2], F32, tag="B7")
    for ch in range(2):
        bnT = wp.tile([128, 4], F32, tag=f"bnT7_{ch}")
        tload(bnT[:], w_d["bn7"][:, ch * 128:(ch + 1) * 128].rearrange("f c -> c f"))
        A, B = _bn_affine(nc, wp, bnT, f"bn7_{ch}")
        # fold l2b: B7' = A7*l2b + B7
        l2bT = wp.tile([128, 1], F32, tag=f"l2bT{ch}")
        tload(l2bT[:], w_d["l2b"][ch * 128:(ch + 1) * 128].rearrange("(p o) -> p o", o=1))
        t = wp.tile([128, 1], F32, tag=f"b7f{ch}")
        nc.vector.tensor_tensor(out=t[:], in0=A[:], in1=l2bT[:], op=ALU.mult)
        nc.vector.tensor_tensor(out=t[:], in0=B[:], in1=t[:], op=ALU.add)
        nc.vector.tensor_copy(out=A7[:, ch:ch + 1], in_=A[:])
        nc.vector.tensor_copy(out=B7[:, ch:ch + 1], in_=t[:])
    l3bT = wp.tile([40, 1], F32, tag="l3bT")
    tload(l3bT[:], w_d["l3b"][:].rearrange("(p o) -> p o", o=1))

    # ---------------- weight transpose machinery ----------------
    def nat_load(src_ap, rows, cols, col_off=0):
        """DMA a natural-layout [rows, cols] block into the staging ring."""
        t = g.nat.tile([128, 2048], F32, tag="nat")
        tload(t[0:rows, col_off:col_off + cols], src_ap)
        return t

    def pe_t(dst_ap, src_ap, rows):
        """dst[cols, rows] = src[rows, cols]^T via PE + ACT copy."""
        ps = g.psW.tile([128, 128], F32, tag="wtp")
        cols = src_ap.shape[-1]
        nc.tensor.transpose(ps[0:cols, 0:rows], src_ap, ident[0:rows, 0:rows])
        nc.scalar.activation(out=dst_ap, in_=ps[0:cols, 0:rows], func=AF.Copy)

    wn = [wn1]
    wxm = [wxm1]

    def prep_w2():
        t = nat_load(w_d["w2"][:], 64, 128)
        wn2 = wp.tile([64, 64], F32, tag="wn2")
        wxm2 = wp.tile([64, 64], F32, tag="wxm2")
        ps = g.psW.tile([128, 128], F32, tag="wtp")
        nc.tensor.transpose(ps[0:128, 0:64], t[0:64, 0:128], ident[0:64, 0:64])
        nc.scalar.activation(out=wn2[:], in_=ps[0:64, 0:64], func=AF.Copy)
        nc.scalar.activation(out=wxm2[:], in_=ps[64:128, 0:64], func=AF.Copy)
        nc.vector.tensor_tensor(out=wxm2[:], in0=wxm2[:], in1=wn2[:], op=ALU.subtract)
        wn.append(wn2)
        wxm.append(wxm2)

    def prep_w3():
        t = nat_load(w_d["w3"][:], 128, 128)
        wn3 = wp.tile([64, 128], F32, tag="wn3")
        wxm3 = wp.tile([64, 128], F32, tag="wxm3")
        ps = g.psW.tile([128, 128], F32, tag="wtp")
        nc.tensor.transpose(ps[0:128, 0:128], t[0:128, 0:128], ident[:])
        nc.scalar.activation(out=wn3[:], in_=ps[0:64, 0:128], func=AF.Copy)
        nc.scalar.activation(out=wxm3[:], in_=ps[64:128, 0:128], func=AF.Copy)
        nc.vector.tensor_tensor(out=wxm3[:], in0=wxm3[:], in1=wn3[:], op=ALU.subtract)
        wn.append(wn3)
        wxm.append(wxm3)

    def prep_w4():
        t = nat_load(w_d["w4"][0:128, :], 128, 256)
        t2 = nat_load(w_d["w4"][128:256, :], 128, 256)
        wn4 = wp.tile([128, 256], F32, tag="wn4")
        wxm4 = wp.tile([128, 256], F32, tag="wxm4")
        for ob, tt in ((0, t), (1, t2)):
            pe_t(wn4[:, ob * 128:(ob + 1) * 128], tt[0:128, 0:128], 128)
            pe_t(wxm4[:, ob * 128:(ob + 1) * 128], tt[0:128, 128:256], 128)
        nc.vector.tensor_tensor(out=wxm4[:], in0=wxm4[:], in1=wn4[:], op=ALU.subtract)
        wn.append(wn4)
        wxm.append(wxm4)

    w5T = [wp.tile([128, 1024], F32, tag=f"w5T{ci}", name=f"w5T{ci}") for ci in range(4)]

    def prep_w5(half):
        for oi in range(half * 4, half * 4 + 4):
            t = nat_load(w_d["w5"][oi * 128:(oi + 1) * 128, :], 128, 512)
            for ci in range(4):
                pe_t(w5T[ci][:, oi * 128:(oi + 1) * 128],
                     t[0:128, ci * 128:(ci + 1) * 128], 128)

    l1wT = [wp.tile([128, 512], F32, tag=f"l1wT{ci}", name=f"l1wT{ci}") for ci in range(16)]

    def prep_l1w(half):
        for oi in range(half * 2, half * 2 + 2):
            t = nat_load(w_d["l1w"][oi * 128:(oi + 1) * 128, :], 128, 2048)
            for ci in range(16):
                pe_t(l1wT[ci][:, oi * 128:(oi + 1) * 128],
                     t[0:128, ci * 128:(ci + 1) * 128], 128)

    l2wT = [wp.tile([128, 256], F32, tag=f"l2wT{ci}", name=f"l2wT{ci}") for ci in range(4)]

    def prep_l2w():
        for oi in range(2):
            t = nat_load(w_d["l2w"][oi * 128:(oi + 1) * 128, :], 128, 512)
            for ci in range(4):
                pe_t(l2wT[ci][:, oi * 128:(oi + 1) * 128],
                     t[0:128, ci * 128:(ci + 1) * 128], 128)

    l3wT = [wp.tile([128, 40], F32, tag=f"l3wT{ci}", name=f"l3wT{ci}") for ci in range(2)]

    def prep_l3w():
        t = nat_load(w_d["l3w"][:], 40, 256)
        for ci in range(2):
            pe_t(l3wT[ci][:], t[0:40, ci * 128:(ci + 1) * 128], 40)

    # ---------------- per-sample feature tiles ----------------
    # cat layout per sample: catA rows 0:64 = h1, 64:128 = h2; catB = h3;
    # catC/catD = h4 chunks.  All [128, 1024].
    cats = []
    for s in range(2):
        cats.append([g.cat.tile([128, N], F32, tag=f"cat{t}_{s}", name=f"cat{t}_{s}")
                     for t in "ABCD"])
    pooledT = g.cat.tile([128, 32], F32, tag="pooledT")

    # ---------------- edge conv layers (interleave weight prep) ----------------
    fT_in = [[xT[s][:]] + [cats[s][0][0:64, :], cats[s][0][64:128, :], cats[s][1][:]]
             for s in range(2)]
    out_rows = [[[cats[s][0][0:64, :]], [cats[s][0][64:128, :]], [cats[s][1][:]],
                 [cats[s][2][:], cats[s][3][:]]] for s in range(2)]

    weight_prep = {
        (0, 0): prep_w2, (0, 1): prep_w3,
        (1, 0): prep_w4, (1, 1): lambda: prep_w5(0),
        (2, 0): lambda: prep_w5(1), (2, 1): lambda: (prep_l1w(0), prep_l1w(1)),
        (3, 0): lambda: (prep_l2w(), prep_l3w()), (3, 1): lambda: None,
    }
    for li, (C, O) in enumerate(LAYERS):
        for s in range(2):
            edge_conv_layer(g, s, li, C, O, fT_in[s][li],
                            wn[li][:], wxm[li][:], bnAB[li + 1],
                            out_rows[s][li])
            weight_prep[(li, s)]()

    # ---------------- layer 5: 1024-wide conv + pooling ----------------
    for s in range(2):
        catchunks = cats[s]
        for j in range(8):
            h5_ps = g.psS.tile([128, N], F32, tag="score")
            for ci in range(4):
                for f in range(0, N, 512):
                    nc.tensor.matmul(h5_ps[:, f:f + 512],
                                     w5T[ci][:, j * 128:(j + 1) * 128],
                                     catchunks[ci][:, f:f + 512],
                                     start=(ci == 0), stop=(ci == 3))
            h5_sb = g.scp.tile([128, N], F32, tag="scp")
            sums = g.small.tile([128, 1], F32, tag="h5sum")
            nc.scalar.activation(out=h5_sb[:], in_=h5_ps[:], func=AF.Prelu,
                                 bias=B5[:, j:j + 1], scale=A5[:, j:j + 1],
                                 alpha=0.2, accum_out=sums[:])
            # mean -> pooled col (8+j)*2+s ; max -> pooled col j*2+s
            nc.scalar.activation(out=pooledT[:, (8 + j) * 2 + s:(8 + j) * 2 + s + 1],
                                 in_=sums[:], func=AF.Copy, scale=1.0 / N)
            nc.vector.tensor_reduce(out=pooledT[:, j * 2 + s:j * 2 + s + 1],
                                    in_=h5_sb[:], axis=AX.X, op=ALU.max)

    # ---------------- MLP head (both samples as free dim) ----------------
    h6T = g.small.tile([128, 4, 2], F32, tag="h6T")
    for j in range(4):
        h6_ps = g.psU.tile([128, 2], F32, tag="uv")
        for ci in range(16):
            nc.tensor.matmul(h6_ps[:], l1wT[ci][:, j * 128:(j + 1) * 128],
                             pooledT[:, ci * 2:ci * 2 + 2],
                             start=(ci == 0), stop=(ci == 15))
        nc.scalar.activation(out=h6T[:, j, :], in_=h6_ps[:], func=AF.Prelu,
                             bias=B6[:, j:j + 1], scale=A6[:, j:j + 1], alpha=0.2)
    h7T = g.small.tile([128, 2, 2], F32, tag="h7T")
    for j in range(2):
        h7_ps = g.psU.tile([128, 2], F32, tag="uv")
        for ci in range(4):
            nc.tensor.matmul(h7_ps[:], l2wT[ci][:, j * 128:(j + 1) * 128],
                             h6T[:, ci, :], start=(ci == 0), stop=(ci == 3))
        nc.scalar.activation(out=h7T[:, j, :], in_=h7_ps[:], func=AF.Prelu,
                             bias=B7[:, j:j + 1], scale=A7[:, j:j + 1], alpha=0.2)
    out_ps = g.psU.tile([40, 2], F32, tag="uv")
    for ci in range(2):
        nc.tensor.matmul(out_ps[:], l3wT[ci][:], h7T[:, ci, :],
                         start=(ci == 0), stop=(ci == 1))
    out_sb = g.small.tile([40, 2], F32, tag="out")
    nc.vector.tensor_scalar(out=out_sb[:], in0=out_ps[:], scalar1=l3bT[:],
                            scalar2=None, op0=ALU.add)
    nc.sync.dma_start(out=out_d[:], in_=out_sb[:])


def edge_conv_layer(g, s, li, C, O, fT, wnT, wxmT, bnab, out_rows):
    nc = g.nc
    As, Bs = bnab
    noc = (O + 127) // 128

    # ---- prep: xx, aug tiles, u/v ----
    sq = g.sqp.tile([C, N], F32, tag="sq")
    nc.scalar.activation(out=sq[:], in_=fT, func=AF.Square)
    xx = g.sqp.tile([1, N], F32, tag="xx")
    nc.gpsimd.tensor_reduce(out=xx[:], in_=sq[:], axis=AX.C, op=ALU.add)

    if C < 128:
        AUGP = 32 if C < 32 else C
        rhs_aug = g.aug.tile([AUGP + 1, N], F32, tag="rhsaug")
        lhs_aug = g.aug.tile([AUGP + 1, N], F32, tag="lhsaug")
        if AUGP != C:
            nc.gpsimd.memset(rhs_aug[:], 0.0)
            nc.gpsimd.memset(lhs_aug[:], 0.0)
        nc.scalar.activation(out=rhs_aug[0:C, :], in_=fT, func=AF.Copy)
        nc.scalar.activation(out=lhs_aug[0:C, :], in_=fT, func=AF.Copy)
        nc.scalar.activation(out=rhs_aug[AUGP:AUGP + 1, :], in_=xx[:], func=AF.Copy)
        nc.vector.memset(lhs_aug[AUGP:AUGP + 1, :], -0.5)
        fT0 = rhs_aug[0:C, :]
    else:
        rhs_aug = lhs_aug = None
        fT0 = fT

    # u = Wn @ f, v = (Wx-Wn) @ f   (transposed [O, N])
    uT_sb, vT_sb = [], []
    for oc in range(noc):
        ocw = min(128, O - oc * 128)
        ups = g.psU.tile([ocw, N], F32, tag="uv")
        for f in range(0, N, 512):
            nc.tensor.matmul(ups[:, f:f + 512], wnT[:, oc * 128:oc * 128 + ocw],
                             fT0[:, f:f + 512], start=True, stop=True)
        ut = g.feat.tile([ocw, N], F32, tag=f"u{oc}")
        nc.scalar.activation(out=ut[:], in_=ups[:], func=AF.Copy)
        uT_sb.append(ut)
        vps = g.psU.tile([ocw, N], F32, tag="uv")
        for f in range(0, N, 512):
            nc.tensor.matmul(vps[:, f:f + 512], wxmT[:, oc * 128:oc * 128 + ocw],
                             fT0[:, f:f + 512], start=True, stop=True)
        vt = g.feat.tile([ocw, N], F32, tag=f"v{oc}")
        nc.scalar.activation(out=vt[:], in_=vps[:], func=AF.Copy)
        vT_sb.append(vt)

    # ---- pipelined per-b-tile units ----
    st = {}

    def stage1(b):
        sc_ps = g.psS.tile([128, N], F32, tag="score")
        for f in range(0, N, 512):
            if C < 128:
                nc.tensor.matmul(sc_ps[:, f:f + 512],
                                 lhs_aug[:, b * 128:(b + 1) * 128],
                                 rhs_aug[:, f:f + 512], start=True, stop=True)
            else:
                nc.tensor.matmul(sc_ps[:, f:f + 512], fT[:, b * 128:(b + 1) * 128],
                                 fT[:, f:f + 512], start=True, stop=False)
                nc.tensor.matmul(sc_ps[:, f:f + 512], g.cm05[:],
                                 xx[:, f:f + 512], start=False, stop=True)
        packed = g.scp.tile([128, N], U32, tag="scp")
        _stt_u32(nc.vector, nc, packed[:], sc_ps[:].bitcast(U32), 0xFFFFFC00,
                 g.iota[:], ALU.bitwise_and, ALU.bitwise_or)
        st[b] = packed

    def stage2(b):
        packed = st[b]
        packf = packed[:].bitcast(F32)
        top24 = g.small.tile([128, 24], F32, tag="top24")
        nc.vector.max(top24[:, 0:8], packf)
        nc.vector.match_replace(packf, top24[:, 0:8], packf, imm_value=NEG)
        nc.vector.max(top24[:, 8:16], packf)
        nc.vector.match_replace(packf, top24[:, 8:16], packf, imm_value=NEG)
        nc.vector.max(top24[:, 16:24], packf)
        idx32 = g.small.tile([128, 32], U32, tag="idx32")
        _ts_u32(nc.vector, nc, idx32[:, 0:24], top24[:].bitcast(U32), 0x3FF,
                ALU.bitwise_and)
        nc.vector.tensor_copy(out=idx32[:, 24:32], in_=idx32[:, 0:8])
        Xf = g.small.tile([128, 32], F32, tag="Xf")
        nc.vector.tensor_copy(out=Xf[:], in_=idx32[:])
        idxb = g.psI.tile([128, 384], F32, tag="idxb")
        nc.tensor.transpose(idxb[0:32, 0:128], Xf[:], g.ident[:])
        Xt_sb = g.small.tile([32, 128], F32, tag="Xt")
        nc.scalar.activation(out=Xt_sb[:], in_=idxb[0:32, 0:128], func=AF.Copy)
        for j0 in range(2):
            nc.tensor.matmul(idxb[:, 128 + j0 * 128:256 + j0 * 128],
                             g.Bsel[j0][:], Xt_sb[:], start=True, stop=True)
        Y_sb = g.small.tile([128, 256], I16, tag="Y")
        nc.vector.tensor_copy(out=Y_sb[:], in_=idxb[:, 128:384])
        st[b] = Y_sb

    def stage3(b):
        Y_sb = st.pop(b)
        for oc in range(noc):
            ocw = min(128, O - oc * 128)
            gt = []
            for j0 in range(2):
                gg = g.gbuf.tile([ocw, 2048], F32, tag=f"g{j0}", name=f"g{j0}")
                nc.gpsimd.ap_gather(gg[:], uT_sb[oc][:],
                                    Y_sb[0:ocw, j0 * 128:(j0 + 1) * 128],
                                    channels=ocw, num_elems=N, d=1, num_idxs=2048)
                gt.append(gg)
            tmpA = g.small.tile([ocw, 128], F32, tag="tmpA")
            tmpB = g.small.tile([ocw, 128], F32, tag="tmpB")
            for j0 in range(2):
                gv = gt[j0][:].rearrange("o (n p) -> o n p", n=128, p=16)[:, :, 0:10]
                nc.vector.tensor_reduce(out=(tmpA if j0 == 0 else tmpB)[:],
                                        in_=gv, axis=AX.X, op=ALU.max)
            nc.vector.tensor_tensor(out=tmpA[:], in0=tmpA[:], in1=tmpB[:],
                                    op=ALU.max)
            nc.vector.tensor_tensor(out=tmpA[:], in0=tmpA[:],
                                    in1=vT_sb[oc][:, b * 128:(b + 1) * 128],
                                    op=ALU.add)
            nc.scalar.activation(out=out_rows[oc][:, b * 128:(b + 1) * 128],
                                 in_=tmpA[:], func=AF.Prelu,
                                 bias=Bs[oc][:], scale=As[oc][:], alpha=0.2)

    for k in range(10):
        if k < 8:
            stage1(k)
        if 0 <= k - 2:
            stage3(k - 2)
        if 0 <= k - 1 < 8:
            stage2(k - 1)


_NC_CACHE = []


def kernel(**inputs):
    """Full-batch entry: shard 16 samples over 8 cores (2 each), run SPMD."""
    from concourse.bass_utils import run_bass_kernel_spmd

    if not _NC_CACHE:
        _NC_CACHE.append(build_nc())
    nc = _NC_CACHE[0]

    x = np.ascontiguousarray(inputs["x"], dtype=np.float32)
    base = {k: np.ascontiguousarray(v, dtype=np.float32)
            for k, v in inputs.items() if k != "x"}
    cores = list(range(8))
    in_maps = [dict(base, x=np.ascontiguousarray(x[2 * c:2 * c + 2])) for c in cores]
    res = run_bass_kernel_spmd(nc, in_maps, cores).results
    out = np.concatenate([np.ascontiguousarray(r["outT"]).T for r in res], axis=0)
    return out.astype(np.float32)


# revision 9
# speedup vs baseline: 1.1715x; 1.1072x over previous
"""DGCNN classifier forward pass on 8 Trainium2 NeuronCores (Bass/Tile).

Data-parallel over batch: 2 point clouds per core. Per sample:
  4 EdgeConv layers, each:
    - kNN scores via one augmented matmul: score[n,m] = <f_n,f_m> - ||f_m||^2/2
      (rank-equivalent to the reference's pairwise-distance top-k)
    - top-20 per row on the DVE via MAX8/MATCH_REPLACE cascades over
      index-packed scores (column index injected into the low 10 mantissa bits)
    - index redistribution for ap_gather done fully on-chip: PE transpose of
      the [128 rows, 32 slots] index tile + two 0/1 selection matmuls that
      place slot (2*(p%16)+j0) of every row on partition p, so each GPSIMD
      core sees all of its rows' neighbor indices in its own 16 partitions.
    - neighbor max-aggregation via GPSIMD ap_gather over u = Wn @ f, using
      monotonicity of the (positive-gamma) BN + LeakyReLU to commute max:
      h = lrelu(A*(max_k u[idx] + (Wx-Wn) @ f) + B)
  then the 1024-wide conv + max/mean pooling and the 3-layer MLP head.

Weights are DMA'd in natural (contiguous) layout and transposed on-chip by
the tensor engine; per-layer work is software-pipelined (3 stages per
128-row tile) to keep the DVE cascade saturated.
"""
import numpy as np
from contextlib import ExitStack

import concourse.bass as bass
import concourse.bacc as bacc
import concourse.mybir as mybir
from concourse import tile
from concourse import masks

F32 = mybir.dt.float32
U32 = mybir.dt.uint32
U16 = mybir.dt.uint16
I16 = mybir.dt.int16
AF = mybir.ActivationFunctionType
ALU = mybir.AluOpType
AX = mybir.AxisListType

N = 1024
K = 20
EPS = 1e-5
NEG = -3.0e38
LAYERS = [(3, 64), (64, 64), (64, 128), (128, 256)]


def build_nc():
    nc = bacc.Bacc("TRN2", target_bir_lowering=False, debug=False)

    x_d = nc.dram_tensor("x", [2, 3, N], F32, kind="ExternalInput")
    w_d = {}
    for name, shape in [("w1", (64, 6)), ("w2", (64, 128)), ("w3", (128, 128)),
                        ("w4", (256, 256)), ("w5", (1024, 512)),
                        ("l1w", (512, 2048)), ("l2w", (256, 512)), ("l3w", (40, 256)),
                        ("l2b", (256,)), ("l3b", (40,))]:
        w_d[name] = nc.dram_tensor(name, list(shape), F32, kind="ExternalInput")
    for i, c in zip(range(1, 8), [64, 64, 128, 256, 1024, 512, 256]):
        w_d["bn%d" % i] = nc.dram_tensor("bn%d" % i, [4, c], F32, kind="ExternalInput")
    out_d = nc.dram_tensor("outT", [40, 2], F32, kind="ExternalOutput")

    with tile.TileContext(nc) as tc, ExitStack() as ctx:
        emit(nc, tc, ctx, x_d, w_d, out_d)
    nc.compile()
    return nc


def _stt_u32(eng, nc, out, in0, imm, in1, op0, op1):
    """scalar_tensor_tensor with a uint32-typed immediate (bitwise-safe)."""
    return eng.add_instruction(mybir.InstTensorScalarPtr(
        name=nc.get_next_instruction_name(),
        is_scalar_tensor_tensor=True,
        op0=op0, op1=op1,
        ins=[eng.lower_ap(in0),
             mybir.ImmediateValue(dtype=U32, value=imm),
             eng.lower_ap(in1)],
        outs=[eng.lower_ap(out)],
    ))


def _ts_u32(eng, nc, out, in0, imm, op0):
    """tensor_scalar with a uint32-typed immediate."""
    return eng.add_instruction(mybir.InstTensorScalarPtr(
        name=nc.get_next_instruction_name(),
        op0=op0, op1=ALU.bypass,
        ins=[eng.lower_ap(in0),
             mybir.ImmediateValue(dtype=U32, value=imm)],
        outs=[eng.lower_ap(out)],
    ))


def _bn_affine(nc, pool, bnT, tag):
    """bnT: [C<=128, 4] tile AP (cols g,b,m,v) -> (A, B) [C,1] tiles."""
    Cc = bnT.shape[0]
    A = pool.tile([Cc, 1], F32, tag=tag + "A")
    B = pool.tile([Cc, 1], F32, tag=tag + "B")
    t = pool.tile([Cc, 1], F32, tag=tag + "t")
    nc.vector.tensor_scalar(out=t[:], in0=bnT[:, 3:4], scalar1=EPS, scalar2=None,
                            op0=ALU.add)
    nc.vector.reciprocal(out=t[:], in_=t[:])
    nc.scalar.activation(out=t[:], in_=t[:], func=AF.Sqrt)
    nc.vector.tensor_tensor(out=A[:], in0=bnT[:, 0:1], in1=t[:], op=ALU.mult)
    nc.vector.tensor_tensor(out=t[:], in0=bnT[:, 2:3], in1=A[:], op=ALU.mult)
    nc.vector.tensor_tensor(out=B[:], in0=bnT[:, 1:2], in1=t[:], op=ALU.subtract)
    return A, B


class Ctx:
    pass


def emit(nc, tc, ctx, x_d, w_d, out_d):
    g = Ctx()
    g.nc = nc
    g.wp = ctx.enter_context(tc.tile_pool(name="wp", bufs=1))
    g.nat = ctx.enter_context(tc.tile_pool(name="nat", bufs=2))
    g.cat = ctx.enter_context(tc.tile_pool(name="cat", bufs=1))
    g.feat = ctx.enter_context(tc.tile_pool(name="feat", bufs=2))
    g.aug = ctx.enter_context(tc.tile_pool(name="aug", bufs=1))
    g.sqp = ctx.enter_context(tc.tile_pool(name="sqp", bufs=1))
    g.scp = ctx.enter_context(tc.tile_pool(name="scp", bufs=2))
    g.gbuf = ctx.enter_context(tc.tile_pool(name="gbuf", bufs=1))
    g.small = ctx.enter_context(tc.tile_pool(name="small", bufs=2))
    g.psS = ctx.enter_context(tc.tile_pool(name="psS", bufs=2, space="PSUM"))
    g.psU = ctx.enter_context(tc.tile_pool(name="psU", bufs=1, space="PSUM"))
    g.psI = ctx.enter_context(tc.tile_pool(name="psI", bufs=1, space="PSUM"))
    g.psW = ctx.enter_context(tc.tile_pool(name="psW", bufs=1, space="PSUM"))
    wp = g.wp

    # ---------------- constants ----------------
    iota = wp.tile([128, N], U32, tag="iota")
    nc.gpsimd.iota(iota[:], pattern=[[1, N]], base=0, channel_multiplier=0)
    ident = wp.tile([128, 128], F32, tag="ident")
    masks.make_identity(nc, ident[:])
    m05 = wp.tile([128, 128], F32, tag="m05")
    nc.vector.memset(m05[:], -0.5)
    # selection matrices B_j0 [32, 128]: B[k, p] = (k == 2*(p%16)+j0)
    rowk = wp.tile([32, 128], U32, tag="rowk")
    nc.gpsimd.iota(rowk[:], pattern=[[0, 128]], base=0, channel_multiplier=1)
    g.Bsel = []
    for j0 in range(2):
        colv = wp.tile([32, 128], U32, tag=f"colv{j0}")
        nc.gpsimd.iota(colv[:], pattern=[[0, 8], [2, 16]], base=j0,
                       channel_multiplier=0)
        Bj = wp.tile([32, 128], F32, tag=f"Bsel{j0}")
        nc.vector.tensor_tensor(out=Bj[:], in0=rowk[:], in1=colv[:],
                                op=ALU.is_equal)
        g.Bsel.append(Bj)
    g.iota = iota
    g.ident = ident
    g.m05 = m05

    # ---------------- small DMA loads (sync queue) ----------------
    def tload(dst, src_ap):
        nc.sync.dma_start(out=dst, in_=src_ap)

    xT = []
    for s in range(2):
        t = g.cat.tile([3, N], F32, tag=f"xT{s}")
        tload(t[:], x_d[s])
        xT.append(t)

    # w1 halves: tiny, element-level transpose DMA is fine
    wn1 = wp.tile([3, 64], F32, tag="wn1")
    tload(wn1[:], w_d["w1"][:, 0:3].rearrange("o c -> c o"))
    wx1 = wp.tile([3, 64], F32, tag="wx1")
    tload(wx1[:], w_d["w1"][:, 3:6].rearrange("o c -> c o"))
    wxm1 = wp.tile([3, 64], F32, tag="wxm1")
    nc.vector.tensor_copy(out=wxm1[:], in_=wx1[:])
    nc.vector.tensor_tensor(out=wxm1[:], in0=wxm1[:], in1=wn1[:], op=ALU.subtract)

    # bn params (small transposed loads) + affines
    bnAB = {}
    for i, c in zip(range(1, 5), [64, 64, 128, 256]):
        nch = (c + 127) // 128
        As, Bs = [], []
        for ch in range(nch):
            cc = min(128, c - ch * 128)
            bnT = wp.tile([cc, 4], F32, tag=f"bnT{i}_{ch}")
            tload(bnT[:], w_d["bn%d" % i][:, ch * 128:ch * 128 + cc].rearrange("f c -> c f"))
            A, B = _bn_affine(nc, wp, bnT, f"bn{i}_{ch}")
            As.append(A)
            Bs.append(B)
        bnAB[i] = (As, Bs)
    A5 = wp.tile([128, 8], F32, tag="A5")
    B5 = wp.tile([128, 8], F32, tag="B5")
    for ch in range(8):
        bnT = wp.tile([128, 4], F32, tag=f"bnT5_{ch}")
        tload(bnT[:], w_d["bn5"][:, ch * 128:(ch + 1) * 128].rearrange("f c -> c f"))
        A, B = _bn_affine(nc, wp, bnT, f"bn5_{ch}")
        nc.vector.tensor_copy(out=A5[:, ch:ch + 1], in_=A[:])
        nc.vector.tensor_copy(out=B5[:, ch:ch + 1], in_=B[:])
    A6 = wp.tile([128, 4], F32, tag="A6")
    B6 = wp.tile([128, 4], F32, tag="B6")
    for ch in range(4):
        bnT = wp.tile([128, 4], F32, tag=f"bnT6_{ch}")
        tload(bnT[:], w_d["bn6"][:, ch * 128:(ch + 1) * 128].rearrange("f c -> c f"))
        A, B = _bn_affine(nc, wp, bnT, f"bn6_{ch}")
        nc.vector.tensor_copy(out=A6[:, ch:ch + 1], in_=A[:])
        nc.vector.tensor_copy(out=B6[:, ch:ch + 1], in_=B[:])
    A7 = wp.tile([128, 2], F32, tag="A7")
    B7 = wp.tile([128, 2], F32, tag="B7")
    for ch in range(2):
        bnT = wp.tile([128, 4], F32, tag=f"bnT7_{ch}")
        tload(bnT[:], w_d["bn7"][:, ch * 128:(ch + 1) * 128].rearrange("f c -> c f"))
        A, B = _bn_affine(nc, wp, bnT, f"bn7_{ch}")
        # fold l2b: B7' = A7*l2b + B7
        l2bT = wp.tile([128, 1], F32, tag=f"l2bT{ch}")
        tload(l2bT[:], w_d["l2b"][ch * 128:(ch + 1) * 128].rearrange("(p o) -> p o", o=1))
        t = wp.tile([128, 1], F32, tag=f"b7f{ch}")
        nc.vector.tensor_tensor(out=t[:], in0=A[:], in1=l2bT[:], op=ALU.mult)
        nc.vector.tensor_tensor(out=t[:], in0=B[:], in1=t[:], op=ALU.add)
        nc.vector.tensor_copy(out=A7[:, ch:ch + 1], in_=A[:])
        nc.vector.tensor_copy(out=B7[:, ch:ch + 1], in_=t[:])
    l3bT = wp.tile([40, 1], F32, tag="l3bT")
    tload(l3bT[:], w_d["l3b"][:].rearrange("(p o) -> p o", o=1))

    # ---------------- weight transpose machinery ----------------
    def nat_load(src_ap, rows, cols, col_off=0):
        """DMA a natural-layout [rows, cols] block into the staging ring."""
        t = g.nat.tile([128, 2048], F32, tag="nat")
        tload(t[0:rows, col_off:col_off + cols], src_ap)
        return t

    def pe_t(dst_ap, src_ap, rows):
        """dst[cols, rows] = src[rows, cols]^T via PE + ACT copy."""
        ps = g.psW.tile([128, 128], F32, tag="wtp")
        cols = src_ap.shape[-1]
        nc.tensor.transpose(ps[0:cols, 0:rows], src_ap, ident[0:rows, 0:rows])
        nc.scalar.activation(out=dst_ap, in_=ps[0:cols, 0:rows], func=AF.Copy)

    wn = [wn1]
    wxm = [wxm1]

    def prep_w2():
        t = nat_load(w_d["w2"][:], 64, 128)
        wn2 = wp.tile([64, 64], F32, tag="wn2")
        wxm2 = wp.tile([64, 64], F32, tag="wxm2")
        ps = g.psW.tile([128, 128], F32, tag="wtp")
        nc.tensor.transpose(ps[0:128, 0:64], t[0:64, 0:128], ident[0:64, 0:64])
        nc.scalar.activation(out=wn2[:], in_=ps[0:64, 0:64], func=AF.Copy)
        nc.scalar.activation(out=wxm2[:], in_=ps[64:128, 0:64], func=AF.Copy)
        nc.vector.tensor_tensor(out=wxm2[:], in0=wxm2[:], in1=wn2[:], op=ALU.subtract)
        wn.append(wn2)
        wxm.append(wxm2)

    def prep_w3():
        t = nat_load(w_d["w3"][:], 128, 128)
        wn3 = wp.tile([64, 128], F32, tag="wn3")
        wxm3 = wp.tile([64, 128], F32, tag="wxm3")
        ps = g.psW.tile([128, 128], F32, tag="wtp")
        nc.tensor.transpose(ps[0:128, 0:128], t[0:128, 0:128], ident[:])
        nc.scalar.activation(out=wn3[:], in_=ps[0:64, 0:128], func=AF.Copy)
        nc.scalar.activation(out=wxm3[:], in_=ps[64:128, 0:128], func=AF.Copy)
        nc.vector.tensor_tensor(out=wxm3[:], in0=wxm3[:], in1=wn3[:], op=ALU.subtract)
        wn.append(wn3)
        wxm.append(wxm3)

    def prep_w4():
        t = nat_load(w_d["w4"][0:128, :], 128, 256)
        t2 = nat_load(w_d["w4"][128:256, :], 128, 256)
        wn4 = wp.tile([128, 256], F32, tag="wn4")
        wxm4 = wp.tile([128, 256], F32, tag="wxm4")
        for ob, tt in ((0, t), (1, t2)):
            pe_t(wn4[:, ob * 128:(ob + 1) * 128], tt[0:128, 0:128], 128)
            pe_t(wxm4[:, ob * 128:(ob + 1) * 128], tt[0:128, 128:256], 128)
        nc.vector.tensor_tensor(out=wxm4[:], in0=wxm4[:], in1=wn4[:], op=ALU.subtract)
        wn.append(wn4)
        wxm.append(wxm4)

    w5T = [wp.tile([128, 1024], F32, tag=f"w5T{ci}", name=f"w5T{ci}") for ci in range(4)]

    def prep_w5(half):
        for oi in range(half * 4, half * 4 + 4):
            t = nat_load(w_d["w5"][oi * 128:(oi + 1) * 128, :], 128, 512)
            for ci in range(4):
                pe_t(w5T[ci][:, oi * 128:(oi + 1) * 128],
                     t[0:128, ci * 128:(ci + 1) * 128], 128)

    l1wT = [wp.tile([128, 512], F32, tag=f"l1wT{ci}", name=f"l1wT{ci}") for ci in range(16)]

    def prep_l1w(half):
        for oi in range(half * 2, half * 2 + 2):
            t = nat_load(w_d["l1w"][oi * 128:(oi + 1) * 128, :], 128, 2048)
            for ci in range(16):
                pe_t(l1wT[ci][:, oi * 128:(oi + 1) * 128],
                     t[0:128, ci * 128:(ci + 1) * 128], 128)

    l2wT = [wp.tile([128, 256], F32, tag=f"l2wT{ci}", name=f"l2wT{ci}") for ci in range(4)]

    def prep_l2w():
        for oi in range(2):
            t = nat_load(w_d["l2w"][oi * 128:(oi + 1) * 128, :], 128, 512)
            for ci in range(4):
                pe_t(l2wT[ci][:, oi * 128:(oi + 1) * 128],
                     t[0:128, ci * 128:(ci + 1) * 128], 128)

    l3wT = [wp.tile([128, 40], F32, tag=f"l3wT{ci}", name=f"l3wT{ci}") for ci in range(2)]

    def prep_l3w():
        t = nat_load(w_d["l3w"][:], 40, 256)
        for ci in range(2):
            pe_t(l3wT[ci][:], t[0:40, ci * 128:(ci + 1) * 128], 40)

    # ---------------- per-sample feature tiles ----------------
    # cat layout per sample: catA rows 0:64 = h1, 64:128 = h2; catB = h3;
    # catC/catD = h4 chunks.  All [128, 1024].
    cats = []
    for s in range(2):
        cats.append([g.cat.tile([128, N], F32, tag=f"cat{t}_{s}", name=f"cat{t}_{s}")
                     for t in "ABCD"])
    pooledT = g.cat.tile([128, 32], F32, tag="pooledT")

    # ---------------- edge conv layers (interleave weight prep) ----------------
    fT_in = [[xT[s][:]] + [cats[s][0][0:64, :], cats[s][0][64:128, :], cats[s][1][:]]
             for s in range(2)]
    out_rows = [[[cats[s][0][0:64, :]], [cats[s][0][64:128, :]], [cats[s][1][:]],
                 [cats[s][2][:], cats[s][3][:]]] for s in range(2)]

    weight_prep = {
        (0, 0): prep_w2, (0, 1): prep_w3,
        (1, 0): prep_w4, (1, 1): lambda: prep_w5(0),
        (2, 0): lambda: prep_w5(1), (2, 1): lambda: (prep_l1w(0), prep_l1w(1)),
        (3, 0): lambda: (prep_l2w(), prep_l3w()), (3, 1): lambda: None,
    }
    for li, (C, O) in enumerate(LAYERS):
        for s in range(2):
            edge_conv_layer(g, s, li, C, O, fT_in[s][li],
                            wn[li][:], wxm[li][:], bnAB[li + 1],
                            out_rows[s][li])
            weight_prep[(li, s)]()

    # ---------------- layer 5: 1024-wide conv + pooling ----------------
    for s in range(2):
        catchunks = cats[s]
        for j in range(8):
            h5_ps = g.psS.tile([128, N], F32, tag="score")
            for ci in range(4):
                for f in range(0, N, 512):
                    nc.tensor.matmul(h5_ps[:, f:f + 512],
                                     w5T[ci][:, j * 128:(j + 1) * 128],
                                     catchunks[ci][:, f:f + 512],
                                     start=(ci == 0), stop=(ci == 3))
            h5_sb = g.scp.tile([128, N], F32, tag="scp")
            sums = g.small.tile([128, 1], F32, tag="h5sum")
            nc.scalar.activation(out=h5_sb[:], in_=h5_ps[:], func=AF.Prelu,
                                 bias=B5[:, j:j + 1], scale=A5[:, j:j + 1],
                                 alpha=0.2, accum_out=sums[:])
            # mean -> pooled col (8+j)*2+s ; max -> pooled col j*2+s
            nc.scalar.activation(out=pooledT[:, (8 + j) * 2 + s:(8 + j) * 2 + s + 1],
                                 in_=sums[:], func=AF.Copy, scale=1.0 / N)
            nc.vector.tensor_reduce(out=pooledT[:, j * 2 + s:j * 2 + s + 1],
                                    in_=h5_sb[:], axis=AX.X, op=ALU.max)

    # ---------------- MLP head (both samples as free dim) ----------------
    h6T = g.small.tile([128, 4, 2], F32, tag="h6T")
    for j in range(4):
        h6_ps = g.psU.tile([128, 2], F32, tag="uv")
        for ci in range(16):
            nc.tensor.matmul(h6_ps[:], l1wT[ci][:, j * 128:(j + 1) * 128],
                             pooledT[:, ci * 2:ci * 2 + 2],
                             start=(ci == 0), stop=(ci == 15))
        nc.scalar.activation(out=h6T[:, j, :], in_=h6_ps[:], func=AF.Prelu,
                             bias=B6[:, j:j + 1], scale=A6[:, j:j + 1], alpha=0.2)
    h7T = g.small.tile([128, 2, 2], F32, tag="h7T")
    for j in range(2):
        h7_ps = g.psU.tile([128, 2], F32, tag="uv")
        for ci in range(4):
            nc.tensor.matmul(h7_ps[:], l2wT[ci][:, j * 128:(j + 1) * 128],
                             h6T[:, ci, :], start=(ci == 0), stop=(ci == 3))
        nc.scalar.activation(out=h7T[:, j, :], in_=h7_ps[:], func=AF.Prelu,
                             bias=B7[:, j:j + 1], scale=A7[:, j:j + 1], alpha=0.2)
    out_ps = g.psU.tile([40, 2], F32, tag="uv")
    for ci in range(2):
        nc.tensor.matmul(out_ps[:], l3wT[ci][:], h7T[:, ci, :],
                         start=(ci == 0), stop=(ci == 1))
    out_sb = g.small.tile([40, 2], F32, tag="out")
    nc.vector.tensor_scalar(out=out_sb[:], in0=out_ps[:], scalar1=l3bT[:],
                            scalar2=None, op0=ALU.add)
    nc.sync.dma_start(out=out_d[:], in_=out_sb[:])


def edge_conv_layer(g, s, li, C, O, fT, wnT, wxmT, bnab, out_rows):
    nc = g.nc
    As, Bs = bnab
    noc = (O + 127) // 128

    # ---- prep: sq = f*f (for the -||f_m||^2/2 score term), u/v ----
    sq = g.sqp.tile([C, N], F32, tag="sq")
    nc.scalar.activation(out=sq[:], in_=fT, func=AF.Square)
    if li == 2:
        # h2 lives at partition base 64 of catA; matmul operands must share
        # a base partition, so stage a base-0 copy.
        f0 = g.aug.tile([64, N], F32, tag="f0")
        nc.scalar.activation(out=f0[:], in_=fT, func=AF.Copy)
        fT0 = f0[:]
    else:
        fT0 = fT

    # u = Wn @ f, v = (Wx-Wn) @ f   (transposed [O, N])
    uT_sb, vT_sb = [], []
    for oc in range(noc):
        ocw = min(128, O - oc * 128)
        ups = g.psU.tile([ocw, N], F32, tag="uv")
        for f in range(0, N, 512):
            nc.tensor.matmul(ups[:, f:f + 512], wnT[:, oc * 128:oc * 128 + ocw],
                             fT0[:, f:f + 512], start=True, stop=True)
        ut = g.feat.tile([ocw, N], F32, tag=f"u{oc}")
        nc.scalar.activation(out=ut[:], in_=ups[:], func=AF.Copy)
        uT_sb.append(ut)
        vps = g.psU.tile([ocw, N], F32, tag="uv")
        for f in range(0, N, 512):
            nc.tensor.matmul(vps[:, f:f + 512], wxmT[:, oc * 128:oc * 128 + ocw],
                             fT0[:, f:f + 512], start=True, stop=True)
        vt = g.feat.tile([ocw, N], F32, tag=f"v{oc}")
        nc.scalar.activation(out=vt[:], in_=vps[:], func=AF.Copy)
        vT_sb.append(vt)

    # ---- pipelined per-b-tile units ----
    st = {}

    def stage1(b):
        # score[n, m] = <f_n, f_m> - 0.5 * sum_c f[c, m]^2
        # (second term via constant -0.5 stationary against sq = f*f)
        sc_ps = g.psS.tile([128, N], F32, tag="score")
        for f in range(0, N, 512):
            nc.tensor.matmul(sc_ps[:, f:f + 512], fT0[:, b * 128:(b + 1) * 128],
                             fT0[:, f:f + 512], start=True, stop=False)
            nc.tensor.matmul(sc_ps[:, f:f + 512], g.m05[0:C, :],
                             sq[:, f:f + 512], start=False, stop=True)
        packed = g.scp.tile([128, N], U32, tag="scp")
        _stt_u32(nc.vector, nc, packed[:], sc_ps[:].bitcast(U32), 0xFFFFFC00,
                 g.iota[:], ALU.bitwise_and, ALU.bitwise_or)
        st[b] = packed

    def stage2(b):
        packed = st[b]
        packf = packed[:].bitcast(F32)
        top24 = g.small.tile([128, 24], F32, tag="top24")
        nc.vector.max(top24[:, 0:8], packf)
        nc.vector.match_replace(packf, top24[:, 0:8], packf, imm_value=NEG)
        nc.vector.max(top24[:, 8:16], packf)
        nc.vector.match_replace(packf, top24[:, 8:16], packf, imm_value=NEG)
        nc.vector.max(top24[:, 16:24], packf)
        idx32 = g.small.tile([128, 32], U32, tag="idx32")
        _ts_u32(nc.vector, nc, idx32[:, 0:24], top24[:].bitcast(U32), 0x3FF,
                ALU.bitwise_and)
        nc.vector.tensor_copy(out=idx32[:, 24:32], in_=idx32[:, 0:8])
        Xf = g.small.tile([128, 32], F32, tag="Xf")
        nc.vector.tensor_copy(out=Xf[:], in_=idx32[:])
        idxb = g.psI.tile([128, 384], F32, tag="idxb")
        nc.tensor.transpose(idxb[0:32, 0:128], Xf[:], g.ident[:])
        Xt_sb = g.small.tile([32, 128], F32, tag="Xt")
        nc.scalar.activation(out=Xt_sb[:], in_=idxb[0:32, 0:128], func=AF.Copy)
        for j0 in range(2):
            nc.tensor.matmul(idxb[:, 128 + j0 * 128:256 + j0 * 128],
                             g.Bsel[j0][:], Xt_sb[:], start=True, stop=True)
        Y_sb = g.small.tile([128, 256], I16, tag="Y")
        nc.vector.tensor_copy(out=Y_sb[:], in_=idxb[:, 128:384])
        st[b] = Y_sb

    def stage3(b):
        Y_sb = st.pop(b)
        for oc in range(noc):
            ocw = min(128, O - oc * 128)
            gt = []
            for j0 in range(2):
                gg = g.gbuf.tile([ocw, 2048], F32, tag=f"g{j0}", name=f"g{j0}")
                nc.gpsimd.ap_gather(gg[:], uT_sb[oc][:],
                                    Y_sb[0:ocw, j0 * 128:(j0 + 1) * 128],
                                    channels=ocw, num_elems=N, d=1, num_idxs=2048)
                gt.append(gg)
            tmpA = g.small.tile([ocw, 128], F32, tag="tmpA")
            tmpB = g.small.tile([ocw, 128], F32, tag="tmpB")
            for j0 in range(2):
                gv = gt[j0][:].rearrange("o (n p) -> o n p", n=128, p=16)[:, :, 0:10]
                nc.vector.tensor_reduce(out=(tmpA if j0 == 0 else tmpB)[:],
                                        in_=gv, axis=AX.X, op=ALU.max)
            nc.vector.tensor_tensor(out=tmpA[:], in0=tmpA[:], in1=tmpB[:],
                                    op=ALU.max)
            nc.vector.tensor_tensor(out=tmpA[:], in0=tmpA[:],
                                    in1=vT_sb[oc][:, b * 128:(b + 1) * 128],
                                    op=ALU.add)
            nc.scalar.activation(out=out_rows[oc][:, b * 128:(b + 1) * 128],
                                 in_=tmpA[:], func=AF.Prelu,
                                 bias=Bs[oc][:], scale=As[oc][:], alpha=0.2)

    for k in range(10):
        if k < 8:
            stage1(k)
        if 0 <= k - 2:
            stage3(k - 2)
        if 0 <= k - 1 < 8:
            stage2(k - 1)


_NC_CACHE = []


def kernel(**inputs):
    """Full-batch entry: shard 16 samples over 8 cores (2 each), run SPMD."""
    from concourse.bass_utils import run_bass_kernel_spmd

    if not _NC_CACHE:
        _NC_CACHE.append(build_nc())
    nc = _NC_CACHE[0]

    x = np.ascontiguousarray(inputs["x"], dtype=np.float32)
    base = {k: np.ascontiguousarray(v, dtype=np.float32)
            for k, v in inputs.items() if k != "x"}
    cores = list(range(8))
    in_maps = [dict(base, x=np.ascontiguousarray(x[2 * c:2 * c + 2])) for c in cores]
    res = run_bass_kernel_spmd(nc, in_maps, cores).results
    out = np.concatenate([np.ascontiguousarray(r["outT"]).T for r in res], axis=0)
    return out.astype(np.float32)
